# revision 38
# baseline (speedup 1.0000x reference)
"""MLA + DeepSeekMoE block kernel for Trainium2, 8 NeuronCores.

Sharding:
  - Attention: token-parallel. Core c handles batch element b=c//4 and the
    256-token query block qb=c%4 (input x pre-rotated per core so its
    queries are rows 0:255; keys are permutation-invariant).
  - MoE: expert-parallel with top-2 sparsity. Core c owns routed expert
    e=c and processes only the tokens routed to it (capacity 640 slots,
    measured max load 554): routing gates are AllGathered, the token index
    list is built on-device (sparse_gather with dummy-slot padding), hn
    rows are fetched by indirect DMA from the AllGathered hn, and the
    gated expert outputs are scattered into a zeroed [T, D] buffer that is
    ReduceScattered in 4 feature chunks. The shared MLP (full ED) runs
    locally on each core's own 256 tokens, off the collective path.

Precision: fp16 matmul operands (weights pre-cast on host), fp32 PSUM and
fp32 row-wise math (RMS norm, softmax recip). fp16 is 1 PE cycle/row (vs
4 for fp32) and halves HBM traffic. Weight DMAs are full-width k-bands
(contiguous per partition) for fat packets.
"""

import math
from contextlib import ExitStack

import numpy as np

import concourse.bass as bass
import concourse.bacc as bacc
import concourse.mybir as mybir
import concourse.tile as tile
from concourse.bass import ds, ts
from concourse.bass_utils import run_bass_kernel_spmd
from concourse.masks import make_identity

AF = mybir.ActivationFunctionType
ALU = mybir.AluOpType
F32 = mybir.dt.float32
F16 = mybir.dt.float16
P = 128

FULL_DIMS = dict(B=2, S=1024, D=2048, H=16, HD=128, ROT=64,
                 LQ=1024, LKV=512, ED=1024, E=8, TOPK=2, EPS=1e-6,
                 N_CORES=8)


def _dram_in(dram, name, shape, dt=F32):
    return dram.tile(shape, dt, kind="ExternalInput", name=name, uniquify=False)


class Builder:
    """Builds the single-core SPMD program."""

    def __init__(self, dims):
        self.d = dict(dims)
        d = self.d
        d["T"] = d["B"] * d["S"]
        d["QB"] = d["N_CORES"] // d["B"]          # query blocks per batch elem
        d["TQ"] = d["S"] // d["QB"]                # query tokens per core
        d["HR"] = d["H"] * d["ROT"]
        d["NPAIR"] = d["ROT"] // 2                 # rope pairs per head (32)
        d["HPT"] = P // d["NPAIR"]                 # heads per rope E/O tile (4)
        d["CAP"] = 640                             # routed-expert token capacity
        assert d["S"] % P == 0 and d["TQ"] % P == 0 and d["D"] % P == 0
        assert d["HD"] == P and d["NPAIR"] == 32
        self.nc = bacc.Bacc(None, target_bir_lowering=False, debug=False,
                            num_devices=d["N_CORES"])

    # ---------------- host-side input prep ----------------

    @staticmethod
    def rope_perm(H, ROT):
        """Column permutation grouping rope features per head: the head's 32
        even lanes, then its 32 odd lanes."""
        NP = ROT // 2
        perm = []
        for h in range(H):
            for i in range(NP):
                perm.append(h * ROT + 2 * i)
            for i in range(NP):
                perm.append(h * ROT + 2 * i + 1)
        return np.asarray(perm)

    def make_in_maps(self, inputs):
        """Full (unsharded) numpy inputs -> list of per-core in_maps."""
        d = self.d
        B, S, D, H, ROT = d["B"], d["S"], d["D"], d["H"], d["ROT"]
        TQ, E = d["TQ"], d["E"]
        x = np.asarray(inputs["x"], np.float32)
        perm = self.rope_perm(H, ROT)
        h16 = lambda a: np.ascontiguousarray(np.asarray(a), np.float16)
        f32c = lambda a: np.ascontiguousarray(np.asarray(a), np.float32)
        shared = {
            "w_lq": h16(inputs["w_lq"]),
            "w_lkv": h16(inputs["w_lkv"]),
            "w_q": h16(inputs["w_q"]),
            "w_k": h16(inputs["w_k"]),
            "w_v": h16(inputs["w_v"]),
            "w_qr": h16(np.asarray(inputs["w_qr"])[:, perm]),
            "w_kr": h16(np.asarray(inputs["w_kr"])[:, perm]),
            "w_o": h16(inputs["w_o"]),
            "centT": h16(np.asarray(inputs["centroids"]).T),
            "b_qr": f32c(np.asarray(inputs["b_qr"])[perm][None]),
            "b_kr": f32c(np.asarray(inputs["b_kr"])[perm][None]),
            "b_o": h16(np.asarray(inputs["b_o"])[None]),
            "bs2": f32c(np.asarray(inputs["bs2"])[None]),
            "w_norm1": f32c(np.asarray(inputs["w_norm1"])[None]),
            "w_norm2": f32c(np.asarray(inputs["w_norm2"])[None]),
            "w_moe_norm": f32c(np.asarray(inputs["w_moe_norm"])[None]),
        }
        fkcT = np.asarray(inputs["freqs_cos"]).T  # [NPAIR, S]
        fksT = np.asarray(inputs["freqs_sin"]).T
        # shared-MLP full weights (each core computes shared for OWN tokens)
        shared["ws1"] = h16(inputs["ws1"])
        shared["bs1"] = f32c(np.asarray(inputs["bs1"])[None])
        shared["ws2"] = h16(inputs["ws2"])
        # sparse-gather iota (i+1 in free-major 16-wrap; pad -> dummy idx 0)
        T, CAP = d["T"], d["CAP"]
        iota = np.zeros((16, T // 16 + CAP // 16), np.float32)
        for i in range(T):
            iota[i % 16, i // 16] = i + 1
        iota[:, T // 16:] = 1.0
        shared["iota_pad"] = iota
        # rope even/odd partition-swap permutation (lhsT for a PE matmul)
        NP = d["NPAIR"]
        sw = np.zeros((P, P), np.float16)
        for m_ in range(P):
            src = m_ + NP if (m_ // NP) % 2 == 0 else m_ - NP
            sw[src, m_] = 1.0
        shared["swap_ident"] = sw
        in_maps = []
        for c in range(d["N_CORES"]):
            b, qb = c // d["QB"], c % d["QB"]
            qoff = qb * TQ
            m = dict(shared)
            # rotate so this core's query block is rows 0:TQ; rope tables
            # rotate identically (attention over keys is perm-invariant).
            m["x_kv"] = h16(np.roll(x[b], -qoff, axis=0))
            m["fkcT"] = h16(np.roll(fkcT, -qoff, 1))
            m["fksT"] = h16(np.roll(fksT, -qoff, 1))
            # expert-parallel slices: core c owns expert c
            m["wr1_e"] = h16(inputs["wr1"][c])
            m["wr2_e"] = h16(inputs["wr2"][c])
            m["br1_e"] = f32c(np.asarray(inputs["br1"])[c][None])
            m["br2_e"] = f32c(np.asarray(inputs["br2"])[c][None])
            oh = np.zeros((1, E), np.float32)
            oh[0, c] = 1.0
            m["onehot"] = np.tile(oh, (1, d["T"] // 128))
            in_maps.append(m)
        return in_maps

    # ---------------- device-side helpers ----------------

    def load_w_bands(self, ctx, tc, name, w_dram, K, M, pool=None,
                     queue=None):
        """[K, M] fp16 DRAM -> list of K//128 SBUF tiles [128, M] (one fat
        contiguous DMA each)."""
        nc = self.nc
        eng = queue if queue is not None else nc.sync
        if pool is None:
            pool = ctx.enter_context(tc.tile_pool(name=f"{name}_wp", bufs=1))
        tiles = []
        for k in range(K // P):
            t = pool.tile([P, M], F16, name=f"{name}_{k}")
            eng.dma_start(out=t[:], in_=w_dram[k * P:(k + 1) * P, :])
            tiles.append(t)
        return tiles

    def load_fm_vec(self, pool, name, dram_vec, n):
        """[n] DRAM fp32 vector -> SBUF [128, n//128] feature-major."""
        t = pool.tile([P, n // P], F32, name=name)
        self.nc.sync.dma_start(out=t[:], in_=dram_vec.rearrange("(c p) -> p c", p=P))
        return t

    def load_row(self, pool, name, dram_row, n, dt=F32):
        t = pool.tile([1, n], dt, name=name)
        self.nc.sync.dma_start(out=t[:], in_=dram_row[:])
        return t

    def bcast_row(self, ctx, tc, name, row_ap, N, out_pool=None):
        """[1, N] fp32 row -> [128, N] broadcast via K=1 ones-matmul."""
        nc = self.nc
        if out_pool is None:
            out_pool = ctx.enter_context(tc.tile_pool(name=f"{name}_bcp", bufs=1))
        out = out_pool.tile([P, N], F32, name=name)
        with tc.tile_pool(name=f"{name}_ps", bufs=2, space="PSUM") as pp:
            for n0 in range(0, N, 512):
                nw = min(512, N - n0)
                ps = pp.tile([P, nw], F32, name=f"{name}_ps")
                nc.tensor.matmul(ps[:], lhsT=self.ones_row_f32[0:1, :P],
                                 rhs=row_ap[0:1, n0:n0 + nw],
                                 start=True, stop=True)
                nc.scalar.copy(out[:, n0:n0 + nw], ps[:])
        return out

    def gemm_rs(self, ctx, tc, name, w_tiles, rhs_tiles, M, N, evict,
                max_cells=4, kslices=None):
        """Resident-SBUF gemm: out[m*128+p, n] = sum_k w[k][p?]... computes
        W.T @ rhs with W = vstack(w_tiles) [K, M], rhs = [K, N].

        w_tiles: KC SBUF tiles [128, M] fp16 (k-bands).
        rhs_tiles: KC SBUF tiles [128, N] fp16.
        evict(m, ns, psum_ap, nw): consume psum cell [128, nw].
        """
        nc = self.nc
        KC = len(w_tiles)
        MC = M // P
        NS = (N + 511) // 512
        cells = [(m, ns) for m in range(MC) for ns in range(NS)]
        loc = ExitStack()
        pspool = loc.enter_context(
            tc.tile_pool(name=f"{name}_ps", bufs=2, space="PSUM"))
        for c0 in range(0, len(cells), max_cells):
            wave = cells[c0:c0 + max_cells]
            ps = {}
            for j, cell in enumerate(wave):
                ps[cell] = pspool.tile([P, 512], F32, name=f"{name}_ps{j}")
            for k in range(KC):
                for (m, ns) in wave:
                    nw = min(512, N - ns * 512)
                    rhs = rhs_tiles[k] if kslices is None else kslices(k)
                    nc.tensor.matmul(
                        ps[(m, ns)][:, :nw],
                        lhsT=w_tiles[k][:, m * P:(m + 1) * P],
                        rhs=rhs[:, ns * 512:ns * 512 + nw],
                        start=(k == 0), stop=(k == KC - 1))
            for (m, ns) in wave:
                nw = min(512, N - ns * 512)
                evict(m, ns, ps[(m, ns)][:, :nw], nw)
        loc.close()

    def gemm_stream_tm(self, ctx, tc, name, act_tiles, w_dram, K, NW, NT,
                       evict, bias_row=None):
        """Token-major streamed gemm: out[nt][128, NW] = act.T @ W (+bias).

        act_tiles: KC SBUF [128, NT*128] fp16 (feature-major activations).
        w_dram: [K, NW] fp16; streams full k-bands (one fat DMA each).
        Requires NT * ceil(NW/512) <= 8 PSUM banks.
        evict(nt, ns, psum_ap, nw).
        """
        nc = self.nc
        KC = K // P
        NS = (NW + 511) // 512
        assert NT * NS <= 8
        loc = ExitStack()
        wpool = loc.enter_context(tc.tile_pool(name=f"{name}_w", bufs=3))
        pspool = loc.enter_context(
            tc.tile_pool(name=f"{name}_ps", bufs=1, space="PSUM"))
        ps = {}
        for nt in range(NT):
            for ns in range(NS):
                ps[(nt, ns)] = pspool.tile([P, 512], F32,
                                           name=f"{name}_ps{nt}_{ns}")
        for k in range(KC):
            wb = wpool.tile([P, NW], F16, name=f"{name}_wb")
            nc.sync.dma_start(out=wb[:], in_=w_dram[k * P:(k + 1) * P, :])
            for nt in range(NT):
                for ns in range(NS):
                    nw = min(512, NW - ns * 512)
                    nc.tensor.matmul(
                        ps[(nt, ns)][:, :nw],
                        lhsT=act_tiles[k][:, nt * P:(nt + 1) * P],
                        rhs=wb[:, ns * 512:ns * 512 + nw],
                        start=(k == 0),
                        stop=(k == KC - 1 and bias_row is None))
        if bias_row is not None:
            for nt in range(NT):
                for ns in range(NS):
                    nw = min(512, NW - ns * 512)
                    nc.tensor.matmul(
                        ps[(nt, ns)][:, :nw],
                        lhsT=self.ones_row[0:1, :P],
                        rhs=bias_row[0:1, ns * 512:ns * 512 + nw],
                        start=False, stop=True)
        for nt in range(NT):
            for ns in range(NS):
                nw = min(512, NW - ns * 512)
                evict(nt, ns, ps[(nt, ns)][:, :nw], nw)
        loc.close()

    def rms_tm(self, ctx, tc, name, x_tiles, NF, wvec_bc=None, out_tiles=None,
               out_dt=F32):
        """Token-major RMS norm: out = x * rsqrt(mean(x^2) + eps) (* wvec)."""
        nc = self.nc
        d = self.d
        spool = ctx.enter_context(tc.tile_pool(name=f"{name}_s", bufs=1))
        if out_tiles is None:
            opool = ctx.enter_context(tc.tile_pool(name=f"{name}_o", bufs=1))
            out_tiles = [opool.tile([P, NF], out_dt, name=f"{name}_o{t}")
                         for t in range(len(x_tiles))]
        loc = ExitStack()
        scratch_pool = loc.enter_context(
            tc.tile_pool(name=f"{name}_sc", bufs=2, space="PSUM"))
        NCH = (NF + 511) // 512
        for t, xt in enumerate(x_tiles):
            use_vec = (t % 2 == 1)   # split sumsq work across scalar+vector
            ss = spool.tile([P, 1], F32, name=f"{name}_ss{t}")
            if use_vec:
                sq = spool.tile([P, NF], F32, name=f"{name}_vsq", bufs=2)
                nc.vector.tensor_tensor(sq[:], xt[:], xt[:], op=ALU.mult)
                nc.vector.tensor_reduce(ss[:], sq[:],
                                        axis=mybir.AxisListType.X, op=ALU.add)
            else:
                pp = spool.tile([P, NCH], F32, name=f"{name}_pp", bufs=2)
                for c in range(NCH):
                    nw = min(512, NF - c * 512)
                    sq = scratch_pool.tile([P, 512], F32, name=f"{name}_sq")
                    nc.scalar.activation(sq[:, :nw],
                                         xt[:, c * 512:c * 512 + nw],
                                         AF.Square, accum_out=pp[:, c:c + 1])
                nc.vector.tensor_reduce(ss[:], pp[:],
                                        axis=mybir.AxisListType.X, op=ALU.add)
            ms = spool.tile([P, 1], F32, name=f"{name}_ms{t}")
            nc.vector.tensor_scalar(ms[:], ss[:], 1.0 / NF, d["EPS"],
                                    op0=ALU.mult, op1=ALU.add)
            rec = spool.tile([P, 1], F32, name=f"{name}_rec{t}")
            nc.vector.reciprocal(rec[:], ms[:])
            rr = spool.tile([P, 1], F32, name=f"{name}_rr{t}")
            nc.scalar.activation(rr[:], rec[:], AF.Sqrt)
            if wvec_bc is not None:
                tmp = spool.tile([P, NF], F32, name=f"{name}_tmp", bufs=2)
                nc.scalar.activation(tmp[:], xt[:], AF.Copy, scale=rr[:])
                nc.vector.tensor_tensor(out_tiles[t][:], tmp[:],
                                        wvec_bc[:, :NF], op=ALU.mult)
            elif use_vec:
                nc.vector.tensor_scalar(out_tiles[t][:], xt[:], rr[:], None,
                                        op0=ALU.mult)
            else:
                nc.scalar.activation(out_tiles[t][:], xt[:], AF.Copy,
                                     scale=rr[:])
        loc.close()
        return out_tiles

    def transpose_to_fm(self, ctx, tc, name, tm_tiles, NF, scale_fm=None,
                        out_pool=None, out_dt=F16, out_tiles=None):
        """Token-major fp16 tiles [128, NF] -> feature-major fp16 tiles
        [128, ntok], optionally scaling rows by scale_fm[:, chunk]."""
        nc = self.nc
        NT = len(tm_tiles)
        FC = NF // P
        if out_tiles is None:
            if out_pool is None:
                out_pool = ctx.enter_context(
                    tc.tile_pool(name=f"{name}_out", bufs=1))
            out_tiles = [out_pool.tile([P, NT * P], out_dt, name=f"{name}_{fc}")
                         for fc in range(FC)]
        loc = ExitStack()
        pspool = loc.enter_context(
            tc.tile_pool(name=f"{name}_ps", bufs=3, space="PSUM"))
        group = 4 if NT % 4 == 0 else (2 if NT % 2 == 0 else 1)
        for fc in range(FC):
            for t0 in range(0, NT, group):
                ps = pspool.tile([P, group * P], F16, name=f"{name}_ps")
                for j in range(group):
                    nc.tensor.transpose(
                        ps[:, j * P:(j + 1) * P],
                        tm_tiles[t0 + j][:, fc * P:(fc + 1) * P],
                        self.ident[:])
                dst = out_tiles[fc][:, t0 * P:(t0 + group) * P]
                if scale_fm is None:
                    nc.scalar.copy(dst, ps[:])
                else:
                    nc.scalar.activation(dst, ps[:], AF.Copy,
                                         scale=scale_fm[:, fc:fc + 1])
        loc.close()
        return out_tiles

    def rope(self, ctx, tc, name, tiles, cos_t, sin_t):
        """In-place rope on interleaved-layout feature-major fp16 tiles.
        The even/odd partition swap runs on the PE (permutation matmul) —
        the PE is otherwise idle here, and DMA partition-shifts stall it."""
        nc = self.nc
        loc = ExitStack()
        pool = loc.enter_context(tc.tile_pool(name=f"{name}_tmp", bufs=2))
        pps = loc.enter_context(
            tc.tile_pool(name=f"{name}_ps", bufs=2, space="PSUM"))
        for tl in tiles:
            N = tl.shape[-1]
            ps = pps.tile([P, N], F32, name=f"{name}_ps")
            for n0 in range(0, N, 512):
                nw = min(512, N - n0)
                nc.tensor.matmul(ps[:, n0:n0 + nw], lhsT=self.swap_ident[:],
                                 rhs=tl[:, n0:n0 + nw], start=True, stop=True)
            t1 = pool.tile([P, N], F16, name=f"{name}_t1")
            nc.vector.tensor_tensor(t1[:], tl[:], cos_t[:, :N], op=ALU.mult)
            swp = pool.tile([P, N], F16, name=f"{name}_sw")
            nc.vector.tensor_tensor(swp[:], ps[:], sin_t[:, :N], op=ALU.mult)
            nc.vector.tensor_tensor(tl[:], t1[:], swp[:], op=ALU.add)
        loc.close()

    # ---------------- the program ----------------

    def build(self):
        d = self.d
        nc = self.nc
        B, S, D, H, HD, ROT = d["B"], d["S"], d["D"], d["H"], d["HD"], d["ROT"]
        LQ, LKV, ED, E = d["LQ"], d["LKV"], d["ED"], d["E"]
        TQ, HR, NPAIR, HPT = d["TQ"], d["HR"], d["NPAIR"], d["HPT"]
        T, NCORES = d["T"], d["N_CORES"]
        DC, SC, TC = D // P, S // P, TQ // P
        TTC = T // P                       # token tiles over all T (16)
        scale = 1.0 / math.sqrt(HD + ROT)
        rg = [list(range(NCORES))]

        with tile.TileContext(nc) as tc, ExitStack() as top:
            dram = top.enter_context(tc.tile_pool(name="dram", bufs=1, space="DRAM"))
            di = lambda n, s, dt=F16: _dram_in(dram, n, s, dt)
            x_kv = di("x_kv", [S, D])
            w_lq = di("w_lq", [D, LQ])
            w_lkv = di("w_lkv", [D, LKV])
            w_q = di("w_q", [LQ, H * HD])
            w_k = di("w_k", [LKV, H * HD])
            w_v = di("w_v", [LKV, H * HD])
            w_qr = di("w_qr", [LQ, HR])
            w_kr = di("w_kr", [D, HR])
            w_o = di("w_o", [H * HD, D])
            wr1_e = di("wr1_e", [D, ED])
            wr2_e = di("wr2_e", [ED, D])
            ws1 = di("ws1", [D, ED])
            ws2 = di("ws2", [ED, D])
            centT = di("centT", [D, E])
            b_qr = di("b_qr", [1, HR], F32)
            b_kr = di("b_kr", [1, HR], F32)
            b_o = di("b_o", [1, D])
            br1_e = di("br1_e", [1, ED], F32)
            br2_e = di("br2_e", [1, D], F32)
            bs1 = di("bs1", [1, ED], F32)
            bs2 = di("bs2", [1, D], F32)
            w_norm1 = di("w_norm1", [1, D], F32)
            w_norm2 = di("w_norm2", [1, D], F32)
            w_moe_norm = di("w_moe_norm", [1, D], F32)
            fkcT = di("fkcT", [NPAIR, S])
            fksT = di("fksT", [NPAIR, S])
            CAP = d["CAP"]
            NG = CAP // P                       # gather groups (5)
            FIN = T // 16 + CAP // 16           # sparse-gather input cols
            NRS = 4                             # feature-chunked RS count
            DCH = D // NRS                      # 512 features per RS chunk
            iota_pad = di("iota_pad", [16, FIN], F32)
            swap_ident = di("swap_ident", [P, P])
            y = dram.tile([TQ, D], F32, kind="ExternalOutput", name="y",
                          uniquify=False)
            # collective bounce buffers
            hn_gin = dram.tile([TQ, D], F16, name="hn_gin")
            hn_gout = dram.tile([T, D], F16, name="hn_gout",
                                addr_space="Shared")
            wm_gin = dram.tile([TQ, E], F32, name="wm_gin")
            wm_gout = dram.tile([T, E], F32, name="wm_gout",
                                addr_space="Shared")
            idx_dram = dram.tile([16, CAP // 16], F32, name="idx_dram")
            gate_dram = dram.tile([16, CAP // 16], F32, name="gate_dram")
            wmcol_dram = dram.tile([T, 1], F32, name="wmcol_dram")
            onehot = di("onehot", [1, (T // P) * E], F32)
            # routed-sum scatter buffers (row T = dummy slot target)
            scat = [dram.tile([T + 1, DCH], F16, name=f"scat{j}")
                    for j in range(NRS)]
            rs_out = [dram.tile([TQ, DCH], F16, name=f"rs_out{j}")
                      for j in range(NRS)]

            const = top.enter_context(
                tc.tile_pool(name="const", bufs=1, side="left"))
            self.ident = const.tile([P, P], F16, name="ident")
            make_identity(nc, self.ident)
            self.swap_ident = const.tile([P, P], F16, name="swap_ident")
            nc.sync.dma_start(out=self.swap_ident[:], in_=swap_ident[:])
            self.ones_row = const.tile([1, P], F16, name="ones_row")
            self.ones_row_f32 = const.tile([1, P], F32, name="ones_row_f32")
            ones_col = const.tile([P, 1], F16, name="ones_col")
            with tc.tile_pool(name="onesc", bufs=1, side="left") as onp:
                s1 = onp.tile([1, P], F32, name="ones_s1")
                nc.vector.memset(s1[:], 1.0)
                nc.scalar.copy(self.ones_row[:], s1[:])
                nc.scalar.copy(self.ones_row_f32[:], s1[:])
                s2 = onp.tile([P, 1], F32, name="ones_s2")
                nc.vector.memset(s2[:], 1.0)
                nc.scalar.copy(ones_col[:], s2[:])

            wn1_fm = self.load_fm_vec(const, "wn1_fm", w_norm1[0, :], D)
            bqr_fm = self.load_fm_vec(const, "bqr_fm", b_qr[0, :], HR)
            bkr_fm = self.load_fm_vec(const, "bkr_fm", b_kr[0, :], HR)
            br1_fm = self.load_fm_vec(const, "br1_fm", br1_e[0, :], ED)
            br2_fm = self.load_fm_vec(const, "br2_fm", br2_e[0, :], D)
            bs1_fm = self.load_fm_vec(const, "bs1_fm", bs1[0, :], ED)

            zt = const.tile([P, DCH], F16, name="zt")
            nc.vector.memset(zt[:], 0.0)
            # norm-weight rows + broadcasts (prefetched; PSUM is free now)
            wn2_row = self.load_row(const, "wn2_row", w_norm2, D)
            wmoe_row = self.load_row(const, "wmoe_row", w_moe_norm, D)
            wn2_bc = self.bcast_row(top, tc, "wn2bc", wn2_row, D)
            wmoe_bc = self.bcast_row(top, tc, "wmoebc", wmoe_row, D)

            # long-lived across phases: residual + hn rows (fp16)
            res_pool = top.enter_context(
                tc.tile_pool(name="xres", bufs=1, side="left"))
            x_res = [res_pool.tile([P, D], F16, name=f"x_res{t}")
                     for t in range(TC)]
            hn_own = [res_pool.tile([P, D], F16, name=f"hn_own{t}")
                      for t in range(TC)]

            # ================= Phase A: attention =================
            with ExitStack() as phAD:
                # cos/sin tiles replicated HPT x along partitions (fp16)
                cs_pool = phAD.enter_context(
                    tc.tile_pool(name="cs", bufs=1, side="left"))
                csk_c = cs_pool.tile([P, S], F16, name="csk_c")
                csk_s = cs_pool.tile([P, S], F16, name="csk_s")
                for j in range(HPT):
                    sl = slice(j * NPAIR, (j + 1) * NPAIR)
                    nc.sync.dma_start(out=csk_c[sl, :], in_=fkcT[:])
                    nc.sync.dma_start(out=csk_s[sl, :], in_=fksT[:])
                for j in range(0, HPT, 2):
                    sl = slice(j * NPAIR, (j + 1) * NPAIR)
                    nc.vector.tensor_scalar_mul(csk_s[sl, :], csk_s[sl, :],
                                                -1.0)
                o_pool = phAD.enter_context(
                    tc.tile_pool(name="att", bufs=1, side="right"))
                o_fm = [o_pool.tile([P, TQ], F16, name=f"o_fm{h}")
                        for h in range(H)]
                xq_pool = phAD.enter_context(
                    tc.tile_pool(name="xq", bufs=1, side="right"))
                x_q = [xq_pool.tile([P, D], F16, name=f"x_q{t}")
                       for t in range(TC)]
                for t in range(TC):
                    nc.sync.dma_start(out=x_q[t][:],
                                      in_=x_kv[t * P:(t + 1) * P, :])
                with ExitStack() as phA:
                    # left-side outputs of A1 (outlive h1)
                    krp_pool = phA.enter_context(
                        tc.tile_pool(name="krp", bufs=1, side="left"))
                    kr_tiles = [krp_pool.tile([P, S], F16, name=f"kr{m}")
                                for m in range(HR // P)]
                    ckv_pool = phA.enter_context(
                        tc.tile_pool(name="ckv", bufs=1, side="left"))
                    ckv_fm = [ckv_pool.tile([P, S], F16, name=f"ckv{m}")
                              for m in range(LKV // P)]
                    cq_pool = phA.enter_context(
                        tc.tile_pool(name="cq", bufs=1, side="left"))
                    cq_fm = [cq_pool.tile([P, TQ], F16, name=f"cq{m}")
                             for m in range(LQ // P)]
                    with ExitStack() as phH1:
                        # --- A0: h1 = rms(x) * w1, feature-major fp16 ---
                        h1_pool = phH1.enter_context(
                            tc.tile_pool(name="h1", bufs=1, side="right"))
                        h1_fm = [h1_pool.tile([P, S], F16, name=f"h1t_{fc}")
                                 for fc in range(DC)]
                        with ExitStack() as phX0:
                            xkv_pool = phX0.enter_context(
                                tc.tile_pool(name="xkv", bufs=1, side="right"))
                            x_tm = [xkv_pool.tile([P, D], F16, name=f"x_tm{t}")
                                    for t in range(SC)]
                            for t in range(SC):
                                nc.sync.dma_start(
                                    out=x_tm[t][:],
                                    in_=x_kv[t * P:(t + 1) * P, :])
                            xr_tm = self.rms_tm(phX0, tc, "rmsA", x_tm, D,
                                                out_tiles=x_tm, out_dt=F16)
                            self.transpose_to_fm(
                                phX0, tc, "h1t", xr_tm, D,
                                scale_fm=wn1_fm, out_tiles=h1_fm)
                        # --- A1: latents kr / ckv / cq from h1 ---
                        with ExitStack() as phB:
                            wkr_sb = self.load_w_bands(phB, tc, "wkr", w_kr,
                                                       D, HR)
                            def ev_kr(m, ns, ps, nw):
                                nc.scalar.activation(
                                    kr_tiles[m][:, ns * 512:ns * 512 + nw],
                                    ps, AF.Identity, bias=bkr_fm[:, m:m + 1])
                            self.gemm_rs(phB, tc, "kr", wkr_sb, h1_fm, HR, S,
                                         ev_kr)
                        self.rope(phA, tc, "ropek", kr_tiles, csk_c, csk_s)
                        with ExitStack() as phB2:
                            wlkv_sb = self.load_w_bands(phB2, tc, "wlkv",
                                                        w_lkv, D, LKV)
                            def ev_ckv(m, ns, ps, nw):
                                nc.scalar.copy(
                                    ckv_fm[m][:, ns * 512:ns * 512 + nw], ps)
                            self.gemm_rs(phB2, tc, "ckv", wlkv_sb, h1_fm,
                                         LKV, S, ev_ckv)
                        with ExitStack() as phB3:
                            wlq_sb = self.load_w_bands(phB3, tc, "wlq", w_lq,
                                                       D, LQ)
                            h1q = [t[:, 0:TQ] for t in h1_fm]
                            def ev_cq(m, ns, ps, nw):
                                nc.scalar.copy(
                                    cq_fm[m][:, ns * 512:ns * 512 + nw], ps)
                            self.gemm_rs(phB3, tc, "cq", wlq_sb, h1q, LQ, TQ,
                                         ev_cq)
                    # h1 freed (top of right stack)
                    # h1 freed
                    # --- A2: projections q/qr (TQ) and k/v (S) ---
                    qp_pool = phA.enter_context(
                        tc.tile_pool(name="qp", bufs=1, side="right"))
                    qp_fm = [qp_pool.tile([P, TQ], F16, name=f"qp{m}")
                             for m in range(H)]
                    qr_pool = phA.enter_context(
                        tc.tile_pool(name="qr", bufs=1, side="right"))
                    qr_tiles = [qr_pool.tile([P, TQ], F16, name=f"qr{m}")
                                for m in range(HR // P)]
                    kp_pool = phA.enter_context(
                        tc.tile_pool(name="kp", bufs=1, side="right"))
                    k_fm = [kp_pool.tile([P, S], F16, name=f"kp{m}")
                            for m in range(H)]
                    v_pool = phA.enter_context(
                        tc.tile_pool(name="vp", bufs=1, side="right"))
                    v_tm = [v_pool.tile([P, H * HD], F16, name=f"v{t}")
                            for t in range(SC)]
                    with ExitStack() as phM1:
                        wq_sb = self.load_w_bands(phM1, tc, "wq", w_q, LQ,
                                                  H * HD)
                        def ev_qp(m, ns, ps, nw):
                            nc.scalar.copy(
                                qp_fm[m][:, ns * 512:ns * 512 + nw], ps)
                        self.gemm_rs(phM1, tc, "qp", wq_sb, cq_fm, H * HD, TQ,
                                     ev_qp)
                    with ExitStack() as phM2:
                        wqr_sb = self.load_w_bands(phM2, tc, "wqr", w_qr, LQ,
                                                   HR)
                        def ev_qr(m, ns, ps, nw):
                            nc.scalar.activation(
                                qr_tiles[m][:, ns * 512:ns * 512 + nw], ps,
                                AF.Identity, bias=bqr_fm[:, m:m + 1])
                        self.gemm_rs(phM2, tc, "qr", wqr_sb, cq_fm, HR, TQ,
                                     ev_qr)
                    self.rope(phA, tc, "ropeq", qr_tiles, csk_c[:, 0:TQ],
                              csk_s[:, 0:TQ])
                    with ExitStack() as phM3:
                        wk_sb = self.load_w_bands(phM3, tc, "wk", w_k, LKV,
                                                  H * HD)
                        def ev_kp(m, ns, ps, nw):
                            nc.scalar.copy(
                                k_fm[m][:, ns * 512:ns * 512 + nw], ps)
                        self.gemm_rs(phM3, tc, "kp", wk_sb, ckv_fm, H * HD, S,
                                     ev_kp)
                    with ExitStack() as phM4:
                        # v: token-major [S, H*HD]; lhsT=ckv token slices,
                        # 2 token tiles x 4 n-chunks = 8 PSUM banks per pass
                        wv_sb = self.load_w_bands(phM4, tc, "wv", w_v, LKV,
                                                  H * HD)
                        with tc.tile_pool(name="v_ps", bufs=1,
                                          space="PSUM") as vps:
                            for pair in range(SC // 2):
                                pstiles = {}
                                for j in range(2):
                                    for ns in range(4):
                                        pstiles[(j, ns)] = vps.tile(
                                            [P, 512], F32, name=f"vps{j}{ns}")
                                for k in range(LKV // P):
                                    for j in range(2):
                                        kt = pair * 2 + j
                                        for ns in range(4):
                                            nc.tensor.matmul(
                                                pstiles[(j, ns)][:],
                                                lhsT=ckv_fm[k][:, kt * P:(kt + 1) * P],
                                                rhs=wv_sb[k][:, ns * 512:(ns + 1) * 512],
                                                start=(k == 0),
                                                stop=(k == LKV // P - 1))
                                for j in range(2):
                                    kt = pair * 2 + j
                                    for ns in range(4):
                                        nc.scalar.copy(
                                            v_tm[kt][:, ns * 512:(ns + 1) * 512],
                                            pstiles[(j, ns)][:])
                    # zero the routed-sum scatter buffers (gpsimd queue; the
                    # startup HBM burst is over, the scatters are far away)
                    for j in range(NRS):
                        for r0 in range(0, T, P):
                            nc.gpsimd.dma_start(out=scat[j][r0:r0 + P, :],
                                                in_=zt[:])
                    # --- A3: attention, software-pipelined over heads (PV of
                    # head h issues after scores of head h+1, so the PE never
                    # stalls on the softmax recip/broadcast chain) ---
                    with ExitStack() as phC:
                        exp_pool = phC.enter_context(
                            tc.tile_pool(name="expp", bufs=2, side="left"))
                        ps_sc = phC.enter_context(
                            tc.tile_pool(name="ps_sc", bufs=3, space="PSUM"))
                        ps_den = phC.enter_context(
                            tc.tile_pool(name="ps_den", bufs=2, space="PSUM"))
                        ps_o = phC.enter_context(
                            tc.tile_pool(name="ps_o", bufs=1, space="PSUM"))
                        sm_pool = phC.enter_context(
                            tc.tile_pool(name="smal", bufs=2, side="left"))

                        def scores_head(h):
                            rt, ro = h // 2, (h % 2) * 2 * NPAIR
                            rsl = slice(ro, ro + 2 * NPAIR)
                            expT = []
                            den = ps_den.tile([1, TQ], F32, name="den")
                            for kt in range(SC):
                                ps = ps_sc.tile([P, TQ], F32, name="ps_sc")
                                nc.tensor.matmul(
                                    ps[:],
                                    lhsT=k_fm[h][:, kt * P:(kt + 1) * P],
                                    rhs=qp_fm[h][:], start=True, stop=False)
                                nc.tensor.matmul(
                                    ps[:],
                                    lhsT=kr_tiles[rt][rsl, kt * P:(kt + 1) * P],
                                    rhs=qr_tiles[rt][rsl, :],
                                    start=False, stop=True)
                                et = exp_pool.tile([P, TQ], F16,
                                                   name=f"expT{kt}")
                                nc.scalar.activation(et[:], ps[:], AF.Exp,
                                                     scale=scale)
                                nc.tensor.matmul(den[:], lhsT=ones_col[:],
                                                 rhs=et[:],
                                                 start=(kt == 0),
                                                 stop=(kt == SC - 1))
                                expT.append(et)
                            recip = sm_pool.tile([1, TQ], F16, name="recip")
                            with nc.allow_low_precision(
                                    reason="fp16 recip for matmul rhs"):
                                nc.vector.reciprocal(recip[:], den[:])
                            return expT, recip

                        def pv_head(h, expT, recip):
                            rbc_ps = ps_o.tile([P, TQ], F32, name="rbc_ps")
                            nc.tensor.matmul(rbc_ps[:],
                                             lhsT=self.ones_row[0:1, :P],
                                             rhs=recip[:], start=True,
                                             stop=True)
                            rbc = sm_pool.tile([P, TQ], F32, name="rbc")
                            nc.scalar.copy(rbc[:], rbc_ps[:])
                            ops = ps_o.tile([P, TQ], F32, name="ops")
                            for kt in range(SC):
                                nc.tensor.matmul(
                                    ops[:],
                                    lhsT=v_tm[kt][:, h * HD:(h + 1) * HD],
                                    rhs=expT[kt][:],
                                    start=(kt == 0),
                                    stop=(kt == SC - 1))
                            nc.vector.tensor_tensor(o_fm[h][:], ops[:],
                                                    rbc[:], op=ALU.mult)

                        prev = None
                        for h in range(H):
                            cur = scores_head(h)
                            if prev is not None:
                                pv_head(h - 1, *prev)
                            prev = cur
                        pv_head(H - 1, *prev)
                # phA closed: h1/kr/ckv/cq/qp/qr/kp/v freed
                # --- A4: output projection + residual ---
                with ExitStack() as phD:
                    bo_row = self.load_row(phD.enter_context(
                        tc.tile_pool(name="bo_p", bufs=1, side="right")),
                        "bo_row", b_o, D, dt=F16)
                    def ev_wo(nt, ns, ps, nw):
                        nc.vector.tensor_tensor(
                            x_res[nt][:, ns * 512:ns * 512 + nw], ps,
                            x_q[nt][:, ns * 512:ns * 512 + nw], op=ALU.add)
                    self.gemm_stream_tm(phD, tc, "wo", o_fm, w_o, H * HD, D,
                                        TC, ev_wo, bias_row=bo_row)
            # phAD closed: o_fm, x_q freed

            # ================= Phase B: hn, routing, gathers =================
            # MoE weights now fit: load them (overlaps with hn compute/gather)
            moe_w = top.enter_context(
                tc.tile_pool(name="moe_w", bufs=1, side="left"))
            # scalar DMA queue: the sync queue carries the ws1/ws2 streams
            # which the shared MLP needs first
            wr1_sb = self.load_w_bands(top, tc, "wr1", wr1_e, D, ED,
                                       pool=moe_w, queue=nc.scalar)
            wr2_sb = self.load_w_bands(top, tc, "wr2", wr2_e, ED, D,
                                       pool=moe_w, queue=nc.scalar)
            hnT_pool = top.enter_context(
                tc.tile_pool(name="hnTp", bufs=1, side="left"))
            with ExitStack() as phE0:
                # fused double-RMS: hn = x*w2*wm * r1*r2 with
                # r1 = rsqrt(mean(x^2)+eps), r2 = rsqrt(r1^2*mean((x*w2)^2)+eps)
                spool = phE0.enter_context(tc.tile_pool(name="rm2", bufs=2))
                scr = phE0.enter_context(
                    tc.tile_pool(name="rm2ps", bufs=2, space="PSUM"))
                for t in range(TC):
                    xt = x_res[t]
                    a2 = spool.tile([P, D], F32, name="a2")
                    nc.vector.tensor_tensor(a2[:], xt[:], wn2_bc[:],
                                            op=ALU.mult)
                    pp1 = spool.tile([P, 4], F32, name="pp1")
                    for cch in range(4):
                        sq = scr.tile([P, 512], F32, name="sq")
                        nc.scalar.activation(
                            sq[:], xt[:, cch * 512:(cch + 1) * 512],
                            AF.Square, accum_out=pp1[:, cch:cch + 1])
                    vsq = spool.tile([P, D], F32, name="vsq", bufs=2)
                    nc.vector.tensor_tensor(vsq[:], a2[:], a2[:], op=ALU.mult)
                    s1 = spool.tile([P, 1], F32, name="s1")
                    nc.vector.tensor_reduce(s1[:], pp1[:],
                                            axis=mybir.AxisListType.X,
                                            op=ALU.add)
                    s2 = spool.tile([P, 1], F32, name="s2")
                    nc.vector.tensor_reduce(s2[:], vsq[:],
                                            axis=mybir.AxisListType.X,
                                            op=ALU.add)
                    m1 = spool.tile([P, 1], F32, name="m1i")
                    nc.vector.tensor_scalar(m1[:], s1[:], 1.0 / D, d["EPS"],
                                            op0=ALU.mult, op1=ALU.add)
                    r1sq = spool.tile([P, 1], F32, name="r1sq")
                    nc.vector.reciprocal(r1sq[:], m1[:])   # r1^2
                    m2 = spool.tile([P, 1], F32, name="m2i")
                    nc.vector.tensor_scalar(m2[:], s2[:], 1.0 / D, None,
                                            op0=ALU.mult)
                    nc.vector.tensor_tensor(m2[:], m2[:], r1sq[:],
                                            op=ALU.mult)
                    nc.vector.tensor_scalar(m2[:], m2[:], 1.0, d["EPS"],
                                            op0=ALU.mult, op1=ALU.add)
                    r2sq = spool.tile([P, 1], F32, name="r2sq")
                    nc.vector.reciprocal(r2sq[:], m2[:])
                    rr = spool.tile([P, 1], F32, name="rr")
                    nc.vector.tensor_tensor(rr[:], r1sq[:], r2sq[:],
                                            op=ALU.mult)
                    nc.scalar.activation(rr[:], rr[:], AF.Sqrt)
                    hsc = spool.tile([P, D], F32, name="hsc")
                    nc.scalar.activation(hsc[:], a2[:], AF.Copy, scale=rr[:])
                    nc.vector.tensor_tensor(hn_own[t][:], hsc[:],
                                            wmoe_bc[:], op=ALU.mult)
                # hn gather input: own token-major rows
                for c in range(TC):
                    nc.sync.dma_start(out=hn_gin[c * P:(c + 1) * P, :],
                                      in_=hn_own[c][:])
                # local routing for OWN tokens (softmax + top-2 over experts)
                hnT_own = self.transpose_to_fm(phE0, tc, "hnT", hn_own, D,
                                               out_pool=hnT_pool)
                cpool = phE0.enter_context(tc.tile_pool(name="centp", bufs=1))
                cent_sb = self.load_w_bands(phE0, tc, "cent", centT, D, E,
                                            pool=cpool)
                ps_r = phE0.enter_context(
                    tc.tile_pool(name="ps_r", bufs=2, space="PSUM"))
                rpool = phE0.enter_context(tc.tile_pool(name="rp", bufs=2))
                for t in range(TC):
                    ps = ps_r.tile([P, E], F32, name="ps_r")
                    for k in range(DC):
                        nc.tensor.matmul(
                            ps[:], lhsT=hnT_own[k][:, t * P:(t + 1) * P],
                            rhs=cent_sb[k][:],
                            start=(k == 0), stop=(k == DC - 1))
                    zt = rpool.tile([P, E], F32, name="zt")
                    mx = rpool.tile([P, 1], F32, name="mx")
                    nc.vector.tensor_reduce(mx[:], ps[:],
                                            axis=mybir.AxisListType.X,
                                            op=ALU.max)
                    negm = rpool.tile([P, 1], F32, name="negm")
                    nc.vector.tensor_scalar_mul(negm[:], mx[:], -1.0)
                    ssum = rpool.tile([P, 1], F32, name="ssum")
                    nc.scalar.activation(zt[:], ps[:], AF.Exp,
                                         bias=negm[:], accum_out=ssum[:])
                    rec = rpool.tile([P, 1], F32, name="rec")
                    nc.vector.reciprocal(rec[:], ssum[:])
                    aff = rpool.tile([P, E], F32, name="aff")
                    nc.scalar.activation(aff[:], zt[:], AF.Copy,
                                         scale=rec[:])
                    m1 = rpool.tile([P, 1], F32, name="m1")
                    nc.vector.tensor_reduce(m1[:], aff[:],
                                            axis=mybir.AxisListType.X,
                                            op=ALU.max)
                    mk1 = rpool.tile([P, E], F32, name="mk1")
                    nc.vector.tensor_scalar(mk1[:], aff[:], m1[:], None,
                                            op0=ALU.is_ge)
                    a2 = rpool.tile([P, E], F32, name="a2")
                    nc.vector.scalar_tensor_tensor(
                        a2[:], mk1[:], -1e30, aff[:],
                        op0=ALU.mult, op1=ALU.add)
                    m2 = rpool.tile([P, 1], F32, name="m2")
                    nc.vector.tensor_reduce(m2[:], a2[:],
                                            axis=mybir.AxisListType.X,
                                            op=ALU.max)
                    mk2 = rpool.tile([P, E], F32, name="mk2")
                    nc.vector.tensor_scalar(mk2[:], a2[:], m2[:], None,
                                            op0=ALU.is_ge)
                    nc.vector.tensor_tensor(mk1[:], mk1[:], mk2[:],
                                            op=ALU.add)
                    wm = rpool.tile([P, E], F32, name="wm")
                    nc.vector.tensor_tensor(wm[:], aff[:], mk1[:],
                                            op=ALU.mult)
                    nc.scalar.dma_start(out=wm_gin[t * P:(t + 1) * P, :],
                                        in_=wm[:])
                # collectives (same order on all cores)
                nc.gpsimd.collective_compute(
                    "AllGather", ALU.bypass, replica_groups=rg,
                    ins=[wm_gin[:].opt()], outs=[wm_gout[:].opt()])
                nc.gpsimd.collective_compute(
                    "AllGather", ALU.bypass, replica_groups=rg,
                    ins=[hn_gin[:].opt()], outs=[hn_gout[:].opt()])

            # ===== Phase C: shared MLP (own tokens) + sparse index build ===
            shg_pool = top.enter_context(
                tc.tile_pool(name="shgp", bufs=1, side="left"))
            shg = [shg_pool.tile([P, D], F32, name=f"shg{t}")
                   for t in range(TC)]
            idx_pool = top.enter_context(
                tc.tile_pool(name="idxp", bufs=1, side="left"))
            idx_i = idx_pool.tile([P, NG], mybir.dt.int32, name="idx_i")
            sidx_i = idx_pool.tile([P, NG], mybir.dt.int32, name="sidx_i")
            gatec = idx_pool.tile([P, NG], F32, name="gatec")
            hng_pool = top.enter_context(
                tc.tile_pool(name="hngp", bufs=1, side="left"))
            hn_g = [hng_pool.tile([P, D], F16, name=f"hng{g}")
                    for g in range(NG)]
            with ExitStack() as phSh:
                # --- sparse index build FIRST (scalar/vector/gpsimd only; the
                # shared-MLP gemms below then hide the whole chain + gathers)
                sgp = phSh.enter_context(tc.tile_pool(name="sgp", bufs=1))
                oh_row = self.load_row(sgp, "oh_row", onehot, TTC * E)
                oh_bc = self.bcast_row(phSh, tc, "ohbc", oh_row, TTC * E,
                                       out_pool=sgp)
                # single-shot gate-column extraction: load the whole gathered
                # gate matrix, multiply by the tiled onehot, reduce per tile
                wm_sb = sgp.tile([P, TTC, E], F32, name="wm_sb")
                nc.scalar.dma_start(
                    out=wm_sb[:],
                    in_=wm_gout.rearrange("(tt p) e -> p tt e", p=P))
                prod = sgp.tile([P, TTC, E], F32, name="prod")
                nc.vector.tensor_tensor(
                    prod.rearrange("p a b -> p (a b)"),
                    wm_sb.rearrange("p a b -> p (a b)"),
                    oh_bc[:, :TTC * E], op=ALU.mult)
                gts = sgp.tile([P, TTC], F32, name="gts")
                nc.vector.tensor_reduce(gts[:], prod[:],
                                        axis=mybir.AxisListType.X, op=ALU.add)
                nc.scalar.dma_start(
                    out=wmcol_dram.rearrange("(tt p) o -> p tt o", p=P),
                    in_=gts[:])
                gcol = sgp.tile([16, T // 16], F32, name="gcol")
                nc.scalar.dma_start(
                    out=gcol[:],
                    in_=wmcol_dram[:, 0:1].rearrange("(f p) o -> p (f o)",
                                                     p=16))
                it16 = sgp.tile([16, FIN], F32, name="it16")
                nc.scalar.dma_start(out=it16[:], in_=iota_pad[:])
                msk = sgp.tile([16, FIN], F32, name="msk")
                nc.vector.memset(msk[:, T // 16:], 1.0)
                nc.vector.tensor_scalar(msk[:, 0:T // 16], gcol[:], 0.0, None,
                                        op0=ALU.is_gt)
                sg1 = sgp.tile([16, FIN], F32, name="sg1")
                nc.vector.tensor_tensor(sg1[:], msk[:], it16[:], op=ALU.mult)
                nc.vector.tensor_scalar(sg1[:], sg1[:], -1.0, None,
                                        op0=ALU.add)
                sg2 = sgp.tile([16, FIN], F32, name="sg2")
                nc.vector.memset(sg2[:, T // 16:], 0.0)
                nc.vector.tensor_tensor(sg2[:, 0:T // 16], gcol[:],
                                        msk[:, 0:T // 16], op=ALU.add)
                nc.vector.tensor_scalar(sg2[:, 0:T // 16], sg2[:, 0:T // 16],
                                        -1.0, None, op0=ALU.add)
                idxf = sgp.tile([16, CAP // 16], F32, name="idxf")
                gatef = sgp.tile([16, CAP // 16], F32, name="gatef")
                nf1 = sgp.tile([1, 1], mybir.dt.uint32, name="nf1")
                nf2 = sgp.tile([1, 1], mybir.dt.uint32, name="nf2")
                nc.gpsimd.sparse_gather(idxf[:], sg1[:], num_found=nf1[:])
                nc.gpsimd.sparse_gather(gatef[:], sg2[:], num_found=nf2[:])
                nc.gpsimd.dma_start(out=idx_dram[:], in_=idxf[:])
                nc.gpsimd.dma_start(out=gate_dram[:], in_=gatef[:])
                # wrap [16, CAP/16] -> [128, NG]: slot = g*128 + p
                idx_col = sgp.tile([P, NG], F32, name="idx_col")
                nc.gpsimd.dma_start(
                    out=idx_col[:],
                    in_=idx_dram.rearrange("pp (g phi) -> phi pp g", phi=8))
                nc.gpsimd.dma_start(
                    out=gatec[:],
                    in_=gate_dram.rearrange("pp (g phi) -> phi pp g", phi=8))
                nc.vector.tensor_copy(idx_i[:], idx_col[:])
                # scatter idx: invalid slots (gate<=0) -> dummy row T
                val = sgp.tile([P, NG], F32, name="val")
                nc.vector.tensor_scalar(val[:], gatec[:], 0.0, None,
                                        op0=ALU.is_gt)
                sx = sgp.tile([P, NG], F32, name="sx")
                nc.vector.tensor_scalar(sx[:], idx_col[:], 1.0, -float(T),
                                        op0=ALU.mult, op1=ALU.add)
                nc.vector.tensor_tensor(sx[:], val[:], sx[:], op=ALU.mult)
                nc.vector.tensor_scalar(sx[:], sx[:], 1.0, float(T),
                                        op0=ALU.mult, op1=ALU.add)
                nc.vector.tensor_copy(sidx_i[:], sx[:])
                # gather hn rows for our expert's tokens (gpsimd queue,
                # overlaps the shared-MLP gemms below)
                for g in range(NG):
                    nc.gpsimd.indirect_dma_start(
                        out=hn_g[g][:], out_offset=None, in_=hn_gout[:],
                        in_offset=bass.IndirectOffsetOnAxis(
                            ap=idx_i[:, g:g + 1], axis=0))

                # shared MLP: u = hn @ ws1 + bs1 (fm), shared = gelu(u@ws2+bs2)
                # gemm1: stream ws1 k-bands; out feature-major [ED, TQ]
                gp = phSh.enter_context(tc.tile_pool(name="gfp", bufs=1))
                g_fm = [gp.tile([P, TQ], F16, name=f"gfm{m}")
                        for m in range(ED // P)]
                ws_pool = phSh.enter_context(tc.tile_pool(name="wsp", bufs=3))
                with tc.tile_pool(name="ps_sh1", bufs=1, space="PSUM") as ps1:
                    pcells = {m: ps1.tile([P, TQ], F32, name=f"ps1_{m}")
                              for m in range(ED // P)}
                    for k in range(DC):
                        wb = ws_pool.tile([P, ED], F16, name="ws1b")
                        nc.sync.dma_start(out=wb[:],
                                          in_=ws1[k * P:(k + 1) * P, :])
                        for m in range(ED // P):
                            nc.tensor.matmul(
                                pcells[m][:],
                                lhsT=wb[:, m * P:(m + 1) * P],
                                rhs=hnT_own[k][:], start=(k == 0),
                                stop=(k == DC - 1))
                    for m in range(ED // P):
                        nc.scalar.activation(g_fm[m][:], pcells[m][:],
                                             AF.Identity,
                                             bias=bs1_fm[:, m:m + 1])
                # gemm2: stream ws2 k-bands; out token-major [TQ, D]
                bs2_row = self.load_row(phSh.enter_context(
                    tc.tile_pool(name="bs2p", bufs=1)), "bs2_row", bs2, D)
                with tc.tile_pool(name="ps_sh2", bufs=1, space="PSUM") as ps2:
                    p2 = {(t, ns): ps2.tile([P, 512], F32,
                                            name=f"ps2_{t}_{ns}")
                          for t in range(TC) for ns in range(4)}
                    for k in range(ED // P):
                        wb = ws_pool.tile([P, D], F16, name="ws2b")
                        nc.sync.dma_start(out=wb[:],
                                          in_=ws2[k * P:(k + 1) * P, :])
                        for t in range(TC):
                            for ns in range(4):
                                nc.tensor.matmul(
                                    p2[(t, ns)][:],
                                    lhsT=g_fm[k][:, t * P:(t + 1) * P],
                                    rhs=wb[:, ns * 512:(ns + 1) * 512],
                                    start=(k == 0), stop=False)
                    for t in range(TC):
                        for ns in range(4):
                            nc.tensor.matmul(
                                p2[(t, ns)][:],
                                lhsT=self.ones_row_f32[0:1, :P],
                                rhs=bs2_row[0:1, ns * 512:(ns + 1) * 512],
                                start=False, stop=True)
                            nc.scalar.activation(
                                shg[t][:, ns * 512:(ns + 1) * 512],
                                p2[(t, ns)][:], AF.Gelu)

            # ===== Phase D: sparse routed experts over CAP slots =========
            with ExitStack() as phMoE:
                # transpose gathered hn rows to feature-major [128, CAP]
                hnf_pool = phMoE.enter_context(
                    tc.tile_pool(name="hnfp", bufs=1, side="right"))
                hn_fm = [hnf_pool.tile([P, CAP], F16, name=f"hnfm{k}")
                         for k in range(DC)]
                with tc.tile_pool(name="htps", bufs=3, space="PSUM") as htps:
                    for k in range(DC):
                        ps = htps.tile([P, CAP], F16, name="htps")
                        for g in range(NG):
                            nc.tensor.transpose(
                                ps[:, g * P:(g + 1) * P],
                                hn_g[g][:, k * P:(k + 1) * P], self.ident[:])
                        nc.scalar.copy(hn_fm[k][:], ps[:])
                # mid = hn @ wr1_e + br1 (fm [ED, CAP])
                mid_pool = phMoE.enter_context(
                    tc.tile_pool(name="midp", bufs=1, side="right"))
                mid_fm = [mid_pool.tile([P, CAP], F16, name=f"mid{m}")
                          for m in range(ED // P)]

                def ev_mid(m, ns, ps, nw):
                    nc.scalar.activation(
                        mid_fm[m][:, ns * 512:ns * 512 + nw], ps,
                        AF.Identity, bias=br1_fm[:, m:m + 1])
                self.gemm_rs(phMoE, tc, "mid", wr1_sb, hn_fm, ED, CAP, ev_mid)

                # out = gelu(mid @ wr2_e + br2) * gate, in NRS feature chunks
                ops2 = phMoE.enter_context(
                    tc.tile_pool(name="ops2", bufs=2, space="PSUM"))
                tps = phMoE.enter_context(
                    tc.tile_pool(name="tps", bufs=2, space="PSUM"))
                ofp = phMoE.enter_context(tc.tile_pool(name="ofp", bufs=2))
                scp = phMoE.enter_context(tc.tile_pool(name="scp", bufs=3))
                MCH = DCH // P                 # fm tiles per RS chunk (4)
                for j in range(NRS):
                    of_fm = []
                    for mi in range(MCH):
                        m = j * MCH + mi
                        pse = {}
                        for ns in range((CAP + 511) // 512):
                            nw = min(512, CAP - ns * 512)
                            pse[ns] = ops2.tile([P, nw], F32, name=f"pse{ns}")
                        for k in range(ED // P):
                            for ns, ph in pse.items():
                                nw = min(512, CAP - ns * 512)
                                nc.tensor.matmul(
                                    ph[:],
                                    lhsT=wr2_sb[k][:, m * P:(m + 1) * P],
                                    rhs=mid_fm[k][:, ns * 512:ns * 512 + nw],
                                    start=(k == 0), stop=(k == ED // P - 1))
                        ot = ofp.tile([P, CAP], F16, name=f"of{mi}")
                        for ns, ph in pse.items():
                            nw = min(512, CAP - ns * 512)
                            nc.scalar.activation(
                                ot[:, ns * 512:ns * 512 + nw], ph[:], AF.Gelu,
                                bias=br2_fm[:, m:m + 1])
                        of_fm.append(ot)
                    # transpose to token-major, scale by gate, scatter
                    for g in range(NG):
                        ps = tps.tile([P, DCH], F16, name="tps")
                        for mi in range(MCH):
                            nc.tensor.transpose(
                                ps[:, mi * P:(mi + 1) * P],
                                of_fm[mi][:, g * P:(g + 1) * P],
                                self.ident[:])
                        sc = scp.tile([P, DCH], F16, name="sc")
                        nc.scalar.activation(sc[:], ps[:], AF.Copy,
                                             scale=gatec[:, g:g + 1])
                        nc.gpsimd.indirect_dma_start(
                            out=scat[j][:], in_=sc[:], in_offset=None,
                            out_offset=bass.IndirectOffsetOnAxis(
                                ap=sidx_i[:, g:g + 1], axis=0))
                    nc.gpsimd.collective_compute(
                        "ReduceScatter", ALU.add, replica_groups=rg,
                        ins=[scat[j][0:T, :].opt()],
                        outs=[rs_out[j][:].opt()])

            # ================= Phase E: final combine =================
            with ExitStack() as phF:
                fpool = phF.enter_context(tc.tile_pool(name="fin", bufs=2))
                for j in range(NRS):
                    cs = slice(j * DCH, (j + 1) * DCH)
                    for t in range(TC):
                        rs_t = fpool.tile([P, DCH], F16, name="rs_t")
                        nc.sync.dma_start(out=rs_t[:],
                                          in_=rs_out[j][t * P:(t + 1) * P, :])
                        ft = fpool.tile([P, DCH], F32, name="fin")
                        nc.vector.tensor_tensor(ft[:], x_res[t][:, cs],
                                                hn_own[t][:, cs], op=ALU.add)
                        nc.vector.tensor_tensor(ft[:], ft[:], shg[t][:, cs],
                                                op=ALU.add)
                        nc.vector.tensor_tensor(ft[:], ft[:], rs_t[:],
                                                op=ALU.add)
                        nc.sync.dma_start(out=y[t * P:(t + 1) * P, cs],
                                          in_=ft[:])

        nc.compile()
        return nc


_CACHE = {}


def _get_built():
    if "nc" not in _CACHE:
        b = Builder(FULL_DIMS)
        _CACHE["builder"] = b
        _CACHE["nc"] = b.build()
    return _CACHE["builder"], _CACHE["nc"]


def kernel(**inputs) -> np.ndarray:
    b, nc = _get_built()
    d = b.d
    in_maps = b.make_in_maps(inputs)
    core_ids = list(range(d["N_CORES"]))
    res = run_bass_kernel_spmd(nc, in_maps, core_ids)
    parts = [res.results[c]["y"] for c in core_ids]
    out = np.concatenate(parts, axis=0).reshape(d["B"], d["S"], d["D"])
    return out.astype(np.float32)

